# revision 2
# baseline (speedup 1.0000x reference)
"""CRF log-partition (forward algorithm) on 8 Trainium2 NeuronCores.

Strategy (data parallel): shard the batch B=128 into 8 shards of 16
columns. Each core runs the T=256 step forward recurrence for its 16
columns in the exp domain:

    state  A[fr=64 partitions, b=16 free]  (scaled alphas, exp domain)
    step:  P[to,b]    = sum_fr exp(trans[to,fr]) * A[fr,b]     (PE matmul)
           A'[to,b]   = P[to,b] * exp(unary[t,b,to] - c[t,b])  (DVE mul)

All 16 lanes run every step unmasked (columns are independent through
the matmul); a lane whose sequence ended keeps evolving into garbage.
Each column's state is captured into Af at its freeze step t = len-1
with an off-critical-path predicated copy (ping-pong A buffers give the
capture two steps of slack).  Per-(t,b) tag-max c is folded out of unary
on the host and added back at the end.  Every 16 steps columns are
rescaled by their column sum; log s accumulates into L gated by
"len > t" so L matches each column's scale at capture time.
Final logZ = log(sum_fr Af*exp(trans[END,fr])) + L + sum_{t<len} c.
"""

import numpy as np

T, B, N = 256, 128, 64
START_IDX, END_IDX = 1, 2
NCORES = 8
BC = B // NCORES  # 16 columns per core
G = 2             # column groups per core (pipeline interleave)
GW = BC // G      # group width (8)
RESCALE = 16      # rescale period in steps


def _build_nc():
    import concourse.bacc as bacc
    import concourse.mybir as mybir
    from concourse.tile import TileContext

    f32 = mybir.dt.float32
    u8 = mybir.dt.uint8
    AF = mybir.ActivationFunctionType

    nc = bacc.Bacc(None, target_bir_lowering=False)
    u_d = nc.dram_tensor("u", [N, T * BC], f32, kind="ExternalInput")
    cap_d = nc.dram_tensor("cap", [N, T * BC], u8, kind="ExternalInput")
    mr_d = nc.dram_tensor("mrow", [1, T * BC], u8, kind="ExternalInput")
    e_d = nc.dram_tensor("e", [N, N], f32, kind="ExternalInput")
    w_d = nc.dram_tensor("wend", [N, 1], f32, kind="ExternalInput")
    a0_d = nc.dram_tensor("a0", [N, BC], f32, kind="ExternalInput")
    o_d = nc.dram_tensor("out", [1, BC], f32, kind="ExternalOutput")

    CH = 16                 # number of load/exp chunks
    CW = T * BC // CH       # chunk width in free elems (256)

    with TileContext(nc) as tc:
        with (
            tc.tile_pool(name="big", bufs=1) as big,
            tc.tile_pool(name="work", bufs=3) as work,
            tc.tile_pool(name="pp", bufs=3, space="PSUM") as pp,
        ):
            U = big.tile([N, T * BC], f32, tag="U")
            Cp = big.tile([N, T * BC], u8, tag="Cp")
            Mr = big.tile([1, T * BC], u8, tag="Mr")
            Ue = big.tile([N, T * BC], f32, tag="Ue")
            E = big.tile([N, N], f32, tag="E")
            W = big.tile([N, 1], f32, tag="W")
            A0 = big.tile([N, BC], f32, tag="A0")
            A1 = big.tile([N, BC], f32, tag="A1")
            Af = big.tile([N, BC], f32, tag="Af")
            L = big.tile([1, BC], f32, tag="L")
            ones_c = big.tile([N, 1], f32, tag="oc")
            ones_r = big.tile([1, N], f32, tag="or")
            gout = big.tile([1, BC], f32, tag="g")
            Abufs = [A0, A1]

            nc.sync.dma_start(E[:], e_d[:])
            nc.sync.dma_start(W[:], w_d[:])
            nc.sync.dma_start(A0[:], a0_d[:])
            nc.sync.dma_start(Mr[:], mr_d[:])
            nc.gpsimd.memset(Af[:], 0.0)
            nc.gpsimd.memset(L[:], 0.0)
            nc.gpsimd.memset(ones_c[:], 1.0)
            nc.gpsimd.memset(ones_r[:], 1.0)

            for ch in range(CH):
                sl = slice(ch * CW, (ch + 1) * CW)
                nc.sync.dma_start(U[:, sl], u_d[:, sl])
                nc.sync.dma_start(Cp[:, sl], cap_d[:, sl])
                nc.scalar.activation(Ue[:, sl], U[:, sl], AF.Exp)

            for t in range(T):
                Ain = Abufs[t % 2]
                Aout = Abufs[(t + 1) % 2]
                for g in range(G):
                    bs = slice(g * GW, (g + 1) * GW)
                    ts_ = slice(t * BC + g * GW, t * BC + (g + 1) * GW)
                    P = pp.tile([N, GW], f32, tag=f"P{g}")
                    nc.tensor.matmul(P[:], E[:], Ain[:, bs], start=True, stop=True)
                    nc.vector.tensor_mul(Aout[:, bs], P[:], Ue[:, ts_])
                # off-chain capture of freezing columns (len == t+1)
                nc.vector.copy_predicated(
                    Af[:], Cp[:, t * BC : (t + 1) * BC], Aout[:]
                )

                if (t + 1) % RESCALE == 0 and (t + 1) < T:
                    for g in range(G):
                        bs = slice(g * GW, (g + 1) * GW)
                        mrow = Mr[0:1, (t + 1) * BC + g * GW : (t + 1) * BC + (g + 1) * GW]
                        s = pp.tile([1, GW], f32, tag=f"P{g}")
                        nc.tensor.matmul(s[:], ones_c[:], Aout[:, bs], start=True, stop=True)
                        r = work.tile([1, GW], f32, tag=f"r{g}")
                        nc.vector.reciprocal(r[:], s[:])
                        Rb = pp.tile([N, GW], f32, tag=f"P{g}")
                        nc.tensor.matmul(Rb[:], ones_r[:], r[:], start=True, stop=True)
                        nc.vector.tensor_mul(Aout[:, bs], Aout[:, bs], Rb[:])
                        lg = work.tile([1, GW], f32, tag=f"lg{g}")
                        nc.scalar.activation(lg[:], s[:], AF.Ln)
                        lgm = work.tile([1, GW], f32, tag=f"lgm{g}")
                        nc.gpsimd.memset(lgm[:], 0.0)
                        nc.vector.copy_predicated(lgm[:], mrow, lg[:])
                        nc.vector.tensor_add(L[0:1, bs], L[0:1, bs], lgm[:])

            for g in range(G):
                bs = slice(g * GW, (g + 1) * GW)
                tm = pp.tile([1, GW], f32, tag=f"P{g}")
                nc.tensor.matmul(tm[:], W[:], Af[:, bs], start=True, stop=True)
                lt = work.tile([1, GW], f32, tag=f"lt{g}")
                nc.scalar.activation(lt[:], tm[:], AF.Ln)
                nc.vector.tensor_add(gout[0:1, bs], lt[:], L[0:1, bs])
            nc.sync.dma_start(o_d[:], gout[:])
    nc.finalize()
    return nc


def _prep_core(unary, lengths, cb):
    """Host-side shard prep for one core's 16 columns starting at cb."""
    u2 = unary[:, cb : cb + BC, :]                      # [T, BC, N]
    ln = lengths[cb : cb + BC]                          # [BC]
    c = u2.max(axis=2)                                  # [T, BC]
    up = u2 - c[:, :, None]                             # [T, BC, N]
    U_sb = np.ascontiguousarray(
        up.transpose(1, 0, 2).reshape(N, T * BC, order="F")
    )
    # layout [to, t*BC + b]
    U_sb = np.ascontiguousarray(
        up.transpose(0, 2, 1).transpose(1, 0, 2).reshape(N, T * BC)
    ).astype(np.float32)
    capm = (np.arange(T)[:, None] + 1 == ln[None, :])   # [T, BC] freeze step
    Cap_sb = np.ascontiguousarray(
        np.broadcast_to(capm[:, None, :], (T, N, BC))
        .transpose(1, 0, 2)
        .reshape(N, T * BC)
    ).astype(np.uint8)
    mrow = (np.arange(T)[:, None] < ln[None, :])        # [T, BC] active at t
    Mr_sb = np.ascontiguousarray(mrow.reshape(1, T * BC)).astype(np.uint8)
    mask = mrow
    C = (c * mask).sum(axis=0).astype(np.float32)       # [BC]
    return U_sb, Cap_sb, Mr_sb, C


def _build_in_maps(unary, trans, lengths):
    unary = np.asarray(unary, dtype=np.float32)
    trans = np.asarray(trans, dtype=np.float32)
    lengths = np.asarray(lengths).astype(np.int64)

    E_h = np.ascontiguousarray(np.exp(trans[0].T)).astype(np.float32)   # [fr,to]
    w_h = np.ascontiguousarray(np.exp(trans[0, END_IDX, :])[:, None]).astype(np.float32)
    a0_h = np.zeros((N, BC), dtype=np.float32)
    a0_h[START_IDX, :] = 1.0

    in_maps, Cs = [], []
    for core in range(NCORES):
        U_sb, Cap_sb, Mr_sb, C = _prep_core(unary, lengths, core * BC)
        in_maps.append(
            {"u": U_sb, "cap": Cap_sb, "mrow": Mr_sb, "e": E_h, "wend": w_h, "a0": a0_h}
        )
        Cs.append(C)
    return in_maps, Cs


def kernel(unary, trans, lengths):
    from concourse.bass_utils import run_bass_kernel_spmd

    in_maps, Cs = _build_in_maps(unary, trans, lengths)
    nc = _build_nc()
    res = run_bass_kernel_spmd(nc, in_maps, list(range(NCORES)))
    outs = [
        res.results[i]["out"].reshape(BC).astype(np.float32) + Cs[i]
        for i in range(NCORES)
    ]
    return np.concatenate(outs).astype(np.float32)

